# revision 11
# baseline (speedup 1.0000x reference)
"""Trainium2 Bass kernel for BitNet multi-head attention (nn_MultiHeadAttention_62294205661880).

Sharding: 8 cores = 2 batches x 4 head-groups (4 heads each).  Each core
computes qkv projection, RoPE, causal attention and a column-parallel slice
of the output projection for its (batch, head-group); the host sums the 4
partial out-projections per batch.

v3: fp8 DoubleRow everywhere except the first 128 queries.  fp8 quantization
error concentrates in the earliest rows (few keys -> no averaging), so rows
0:128 take a bf16 "clean" path while everything else runs fp8 DoubleRow
matmuls (2 contraction blocks per PE pass):
  - Q/K projection: output cols 0:128 from bf16 x, cols 128:2048 DoubleRow
    from fp8 x.
  - V projection: key block 0 from bf16 x (stored bf16+fp8), blocks 1:15
    DoubleRow (stored fp8).  wv is pre-scaled by 1/4 so v fits fp8 range
    (max |v| = 240.5 > 240 = trn-e4m3 max).
  - attention: softmax numerator exp() written as fp8 ki-pairs; denominator
    (ones-matmul) and AV both DoubleRow over ki-pairs.  Rows 0:128 of query
    chunk 0 keep a bf16 sub-path (scores block ki0 -> bf16 exp/sums/AV).
  - out-projection: attention output stored fp8, head-pair DoubleRow
    (seq block 0 bf16 from the clean attention output).
Weights are ternary {-1,0,+1} (x0.25 for wv) -- exact in fp8, so DoubleRow
matmuls only add the activation-side quantization noise.  Simulated
rel-err 0.0045 vs the 2e-2 gate.

Input DMAs are split across both hardware DGE queues (sync + scalar
engines) so the projection matmuls are fed at ~2x single-queue bandwidth;
out-projection emission is software-pipelined behind the next chunk's
first score matmuls to hide the aoT dependency chain.

Device layout trick (unchanged): everything is computed transposed.  Q_T/K_T
come out of the projection as [dh, S]; scores are s_T[k, q]; the softmax
denominator sums over the partition (key) dim via an all-ones stationary
matmul; AV produces out_T[dh, q] which feeds the output projection directly.
No on-device transposes.  Softmax skips the max-subtraction: scores are
bounded (~+-2) because the BitNet weight scale is tiny.
"""

import sys
import types

import numpy as np
import ml_dtypes

import concourse.bass as bass
import concourse.mybir as mybir
import concourse.tile as tile
from concourse import bacc
from concourse.bass_utils import run_bass_kernel_spmd

D_MODEL = 2048
N_HEADS = 16
D_HEAD = 128
SEQ = 2048
BATCH = 2
ROPE_BASE = 10000.0

N_CORES = 8
HPC = 4  # heads per core
R_LOCAL = HPC * D_HEAD  # 512 local q (or k, or v) rows per core
MO = D_MODEL // 128  # 16 contraction blocks
MO2 = MO // 2  # 8 contraction block pairs
NKI = SEQ // 128  # 16 key blocks
NQC = SEQ // 512  # 4 query chunks of 512
NSB = SEQ // 128  # 16 seq blocks (v / proj)
CL = 128  # clean (bf16-path) rows

BF16 = mybir.dt.bfloat16
F32 = mybir.dt.float32
NPBF16 = ml_dtypes.bfloat16
NPFP8 = ml_dtypes.float8_e4m3
FP8 = mybir.dt.float8e4
DR = mybir.MatmulPerfMode.DoubleRow

LAST_RESULT = None  # BassKernelResults of the most recent run (for test.py)
_PROG_CACHE = {}
PROFILE = False  # test.py sets True to capture an NTFF profile / HW exec time


def _enable_profiling() -> bool:
    """Install the axon NTFF profile hook glue if the image lacks
    ``antenv.axon_hooks`` (boot degrades silently without it), and skip
    the artifact upload (no bucket access in this container)."""
    try:
        from antenv.axon_hooks import get_axon_ntff_profile_hook  # noqa: F401

        ok = get_axon_ntff_profile_hook() is not None
    except ImportError:
        ok = False
        import antenv

        mod = types.ModuleType("antenv.axon_hooks")
        mod._hook = None
        mod.set_axon_ntff_profile_hook = lambda h: setattr(mod, "_hook", h)
        mod.get_axon_ntff_profile_hook = lambda: mod._hook
        sys.modules["antenv.axon_hooks"] = mod
        antenv.axon_hooks = mod
        try:
            from trn_agent_boot.trn_boot import _ntff_profile_via_ctypes

            hook = _ntff_profile_via_ctypes("/opt/axon/libaxon_pjrt.so")
            if hook is not None:
                mod._hook = hook
                ok = True
        except Exception as e:  # profiling is best-effort
            print(f"ntff profile hook install failed: {e}", file=sys.stderr)
    if ok:
        import concourse.bass_utils as _bu

        _bu.upload_artifacts = lambda tmpdir: tmpdir
    return ok


def _build_program(exp_scale: float) -> bass.Bass:
    nc = bacc.Bacc(None)
    S = SEQ

    xT8_d = nc.dram_tensor("xT8", [D_MODEL, S], FP8, kind="ExternalInput")
    xT16_d = nc.dram_tensor("xT16", [D_MODEL, CL], BF16, kind="ExternalInput")
    wqT_d = nc.dram_tensor("wqT", [D_MODEL, R_LOCAL], FP8, kind="ExternalInput")
    wkT_d = nc.dram_tensor("wkT", [D_MODEL, R_LOCAL], FP8, kind="ExternalInput")
    wvT_d = nc.dram_tensor("wvT", [D_MODEL, R_LOCAL], FP8, kind="ExternalInput")
    woT_d = nc.dram_tensor("woT", [R_LOCAL, D_MODEL], FP8, kind="ExternalInput")
    # cos rows 0:64, sin rows 64:128
    cs_d = nc.dram_tensor("cossinT", [128, S], BF16, kind="ExternalInput")
    # swapped: sin rows 0:64, cos rows 64:128 (keeps TensorTensor base partitions equal)
    sc_d = nc.dram_tensor("sincosT", [128, S], BF16, kind="ExternalInput")
    # transposed diagonal 128x128 mask blocks, side by side (bf16: block 0 only)
    maskd16_d = nc.dram_tensor("maskd16", [128, CL], BF16, kind="ExternalInput")
    maskd8_d = nc.dram_tensor("maskd8", [128, S], FP8, kind="ExternalInput")
    out_d = nc.dram_tensor("out", [S, D_MODEL], BF16, kind="ExternalOutput")

    xT8_v = xT8_d[:].rearrange("(mo p) s -> p mo s", p=128)
    xT16_v = xT16_d[:].rearrange("(mo p) s -> p mo s", p=128)
    wqT_v = wqT_d[:].rearrange("(mo p) r -> p mo r", p=128)
    wkT_v = wkT_d[:].rearrange("(mo p) r -> p mo r", p=128)
    wvT_v = wvT_d[:].rearrange("(mo p) r -> p mo r", p=128)
    woT_v = woT_d[:].rearrange("(h p) o -> p h o", p=128)

    with tile.TileContext(nc) as tc:
        with tc.tile_pool(name="pers", bufs=1) as pers:
            # ---- persistent SBUF tensors (live across both phases) ----
            q_rot = pers.tile([128, HPC, S], BF16, tag="qrot")
            k_rot = pers.tile([128, HPC, S], BF16, tag="krot")
            v8 = pers.tile([128, NKI, R_LOCAL], FP8, tag="v8")
            v16 = pers.tile([128, 1, R_LOCAL], BF16, tag="v16")
            aoT8 = pers.tile([128, HPC, S], FP8, tag="aoT8")
            aoT16 = pers.tile([128, HPC, CL], BF16, tag="aoT16")
            wo8 = pers.tile([128, HPC, D_MODEL], FP8, tag="wo8")
            ones16 = pers.tile([128, 128], BF16, tag="ones16")
            ones8 = pers.tile([128, 2, 128], FP8, tag="ones8")
            warm = pers.tile([128, 1], BF16, tag="warm")
            maskd16 = pers.tile([128, CL], BF16, tag="maskd16")
            maskd8 = pers.tile([128, S], FP8, tag="maskd8")
            nc.vector.memset(ones16[:, :], 1.0)
            nc.vector.memset(ones8[:, :, :], 1.0)
            # load the exp table set first so no ACT table switch happens
            # mid-kernel (Copy lives in every set).
            nc.scalar.activation(
                warm[:, :], ones16[:, 0:1], mybir.ActivationFunctionType.Exp
            )

            # ================= phase A: QKV projection + RoPE =================
            with (
                tc.tile_pool(name="xtp", bufs=1) as xtp,
                tc.tile_pool(name="wp", bufs=1) as wp,
                tc.tile_pool(name="raw", bufs=3) as rawp,
                tc.tile_pool(name="tmp", bufs=2) as tmpp,
                tc.tile_pool(name="psA", bufs=2, space="PSUM") as psA,
            ):
                xt8 = xtp.tile([128, MO, S], FP8, tag="xt8")  # cols 0:128 unused
                xt16 = xtp.tile([128, MO, CL], BF16, tag="xt16")
                wq = wp.tile([128, MO, R_LOCAL], FP8, tag="wq")
                wk = wp.tile([128, MO, R_LOCAL], FP8, tag="wk")
                wv = wp.tile([128, MO, R_LOCAL], FP8, tag="wv")
                cs_t = wp.tile([128, S], BF16, tag="cs")
                sc_t = wp.tile([128, S], BF16, tag="sc")

                # Input DMAs on both hardware DGE queues (~195 GB/s each).
                # Head 0 runs its bf16 clean m-loop first (small xt16, lands
                # early on queue B) while the xt8 pairs stream on both queues
                # interleaved with wq/wk, so the DR m2-outer loop never
                # starves.  queue B (scalar): xt16, even xt8 pairs, cs/sc,
                # wo8.  queue A (sync): wq/wk, odd xt8 pairs, wv, masks.
                def dma_x8(eng, m2):
                    sl = slice(2 * m2, 2 * m2 + 2)
                    if m2 == 0:
                        for c4 in range(4):
                            lo = 128 if c4 == 0 else c4 * 512
                            eng.dma_start(
                                out=xt8[:, sl, lo : (c4 + 1) * 512],
                                in_=xT8_v[:, sl, lo : (c4 + 1) * 512],
                            )
                    else:
                        eng.dma_start(out=xt8[:, sl, 128:S], in_=xT8_v[:, sl, 128:S])

                nc.scalar.dma_start(out=xt16[:, :, :], in_=xT16_v[:, :, :])
                nc.sync.dma_start(out=wq[:, 0:2, :], in_=wqT_v[:, 0:2, :])
                nc.sync.dma_start(out=wk[:, 0:2, :], in_=wkT_v[:, 0:2, :])
                dma_x8(nc.scalar, 0)
                dma_x8(nc.sync, 1)
                dma_x8(nc.sync, 3)
                dma_x8(nc.scalar, 2)
                dma_x8(nc.scalar, 4)
                for m2 in range(1, MO2):
                    sl = slice(2 * m2, 2 * m2 + 2)
                    nc.sync.dma_start(out=wq[:, sl, :], in_=wqT_v[:, sl, :])
                    nc.sync.dma_start(out=wk[:, sl, :], in_=wkT_v[:, sl, :])
                dma_x8(nc.scalar, 6)
                dma_x8(nc.sync, 5)
                dma_x8(nc.sync, 7)
                nc.scalar.dma_start(out=cs_t[:, :], in_=cs_d[:, :])
                nc.scalar.dma_start(out=sc_t[:, :], in_=sc_d[:, :])
                for m2 in range(MO2):
                    sl = slice(2 * m2, 2 * m2 + 2)
                    nc.sync.dma_start(out=wv[:, sl, :], in_=wvT_v[:, sl, :])
                nc.scalar.dma_start(out=wo8[:, :, :], in_=woT_v[:, :, :])
                nc.sync.dma_start(out=maskd16[:, :], in_=maskd16_d[:, :])
                nc.sync.dma_start(out=maskd8[:, :], in_=maskd8_d[:, :])

                def rope(dst, raw):
                    """NeoX rotary: rows 0:64 = t*c - b*s ; rows 64:128 = t*s + b*c."""
                    ta = tmpp.tile([64, S], BF16, tag="tmp")
                    tb = tmpp.tile([64, S], BF16, tag="tmp")
                    nc.vector.tensor_mul(ta[:, :], raw[0:64, :], cs_t[0:64, :])
                    nc.vector.tensor_mul(tb[:, :], raw[64:128, :], cs_t[64:128, :])
                    nc.vector.tensor_sub(dst[0:64, :], ta[:, :], tb[:, :])
                    tc2 = tmpp.tile([64, S], BF16, tag="tmp")
                    td = tmpp.tile([64, S], BF16, tag="tmp")
                    nc.vector.tensor_mul(tc2[:, :], raw[0:64, :], sc_t[0:64, :])
                    nc.vector.tensor_mul(td[:, :], raw[64:128, :], sc_t[64:128, :])
                    nc.vector.tensor_add(dst[64:128, :], tc2[:, :], td[:, :])

                def dr_chunks(ps, w_sb, hsl, msl, first, last):
                    """DoubleRow q/k chunk matmuls for one m-pair (cols 128:2048).

                    The c4=0 chunk shares its PSUM bank (2KB zero region) with
                    the bf16 clean cols 0:128, whose matmuls come later: only
                    the first matmul per bank may carry start=True, and the
                    bank's stop stays with its last writer (the clean loop)."""
                    for c4 in range(4):
                        lo = 128 if c4 == 0 else c4 * 512
                        csl = slice(lo, (c4 + 1) * 512)
                        nc.tensor.matmul(
                            ps[:, csl],
                            w_sb[:, msl, hsl],
                            xt8[:, msl, csl],
                            start=first,
                            stop=last and c4 != 0,
                            perf_mode=DR,
                        )

                # head 0 q/k: bf16 clean m-loop first (xt16 lands first),
                # then the DR m2-outer loop consumes xt8 m-pairs as the DMAs
                # land (startup overlap).  The clean loop owns zone0's
                # start=True; the c4=0 DR chunk carries its stop.
                qp0 = psA.tile([128, S], F32, tag="psA")
                kp0 = psA.tile([128, S], F32, tag="psA")
                for m in range(MO):
                    nc.tensor.matmul(
                        qp0[:, 0:CL],
                        wq[:, m, 0:128],
                        xt16[:, m, :],
                        start=(m == 0),
                        stop=False,
                    )
                    nc.tensor.matmul(
                        kp0[:, 0:CL],
                        wk[:, m, 0:128],
                        xt16[:, m, :],
                        start=(m == 0),
                        stop=False,
                    )
                for m2 in range(MO2):
                    msl = slice(2 * m2, 2 * m2 + 2)
                    for ps, w_sb in ((qp0, wq), (kp0, wk)):
                        for c4 in range(4):
                            lo = 128 if c4 == 0 else c4 * 512
                            nc.tensor.matmul(
                                ps[:, lo : (c4 + 1) * 512],
                                w_sb[:, msl, 0:128],
                                xt8[:, msl, lo : (c4 + 1) * 512],
                                start=(m2 == 0 and c4 != 0),
                                stop=(m2 == MO2 - 1),
                                perf_mode=DR,
                            )
                q_raw = rawp.tile([128, S], BF16, tag="raw")
                nc.scalar.copy(q_raw[:, :], qp0[:, :])
                rope(q_rot[:, 0, :], q_raw)
                k_raw = rawp.tile([128, S], BF16, tag="raw")
                nc.scalar.copy(k_raw[:, :], kp0[:, :])
                rope(k_rot[:, 0, :], k_raw)

                def project(dst_raw, w_sb, h):
                    """q/k head projection -> bf16 raw [128, S] (xt resident)."""
                    hsl = slice(h * 128, (h + 1) * 128)
                    ps = psA.tile([128, S], F32, tag="psA")
                    for m2 in range(MO2):
                        msl = slice(2 * m2, 2 * m2 + 2)
                        dr_chunks(ps, w_sb, hsl, msl, m2 == 0, m2 == MO2 - 1)
                    for m in range(MO):
                        nc.tensor.matmul(
                            ps[:, 0:CL],
                            w_sb[:, m, hsl],
                            xt16[:, m, :],
                            start=False,
                            stop=(m == MO - 1),
                        )
                    nc.scalar.copy(dst_raw[:, :], ps[:, :])

                def v_group(sb4):
                    """V projection group (4 seq blocks, natural layout [s, r]).
                    sb0 (keys 0:128) from bf16 x, stored bf16 + fp8; the rest
                    DoubleRow from fp8 x, stored fp8 only."""
                    ps = psA.tile([128, S], F32, tag="psA")
                    for part in range(4):
                        sb = sb4 * 4 + part
                        ssl = slice(sb * 128, (sb + 1) * 128)
                        psl = slice(part * 512, part * 512 + 512)
                        if sb == 0:
                            for m in range(MO):
                                nc.tensor.matmul(
                                    ps[:, psl],
                                    xt16[:, m, :],
                                    wv[:, m, :],
                                    start=(m == 0),
                                    stop=(m == MO - 1),
                                )
                        else:
                            for m2 in range(MO2):
                                msl = slice(2 * m2, 2 * m2 + 2)
                                nc.tensor.matmul(
                                    ps[:, psl],
                                    xt8[:, msl, ssl],
                                    wv[:, msl, :],
                                    start=(m2 == 0),
                                    stop=(m2 == MO2 - 1),
                                    perf_mode=DR,
                                )
                    k0 = sb4 * 4
                    if sb4 == 0:
                        nc.scalar.copy(v16[:, 0, :], ps[:, 0:512])
                        nc.vector.tensor_copy(v8[:, 0:2, :], ps[:, 0:1024])
                        nc.scalar.copy(v8[:, 2:4, :], ps[:, 1024:2048])
                    else:
                        nc.vector.tensor_copy(
                            v8[:, k0 : k0 + 2, :], ps[:, 0:1024]
                        )
                        nc.scalar.copy(v8[:, k0 + 2 : k0 + 4, :], ps[:, 1024:2048])

                # Interleave V groups between the q/k head projections so the
                # last head's RoPE (DVE) drains while the PE runs V matmuls.
                for h in range(1, HPC):
                    q_raw = rawp.tile([128, S], BF16, tag="raw")
                    project(q_raw, wq, h)
                    rope(q_rot[:, h, :], q_raw)
                    k_raw = rawp.tile([128, S], BF16, tag="raw")
                    project(k_raw, wk, h)
                    rope(k_rot[:, h, :], k_raw)
                    v_group(h - 1)
                v_group(3)

            # ================= phase B: attention + out-projection =============
            with (
                tc.tile_pool(name="pp8", bufs=4) as pp8p,
                tc.tile_pool(name="ppc", bufs=2) as ppcp,
                tc.tile_pool(name="rcp", bufs=3) as rcp,
                tc.tile_pool(name="osb", bufs=4) as osbp,
                tc.tile_pool(name="sp", bufs=2, space="PSUM") as spp,
                tc.tile_pool(name="acc", bufs=2, space="PSUM") as accp,
            ):
                EXP = mybir.ActivationFunctionType.Exp
                esc = float(exp_scale)
                evict_flip = [0]

                def scores_pair(qc, h, kp, pp):
                    """scores + exp + mask for ki-pair kp -> pp [128, 2, 512].
                    Returns q0, the first valid column of the pair."""
                    q_lo = qc * 512
                    ki0, ki1 = 2 * kp, 2 * kp + 1
                    k0sl = slice(ki0 * 128, (ki0 + 1) * 128)
                    k1sl = slice(ki1 * 128, (ki1 + 1) * 128)
                    spb = spp.tile([128, 1024], F32, tag="sp")
                    if kp < 2 * qc:  # off-diagonal pair
                        nc.tensor.matmul(
                            spb[:, 0:512],
                            k_rot[:, h, k0sl],
                            q_rot[:, h, q_lo : q_lo + 512],
                            start=True,
                            stop=True,
                        )
                        nc.tensor.matmul(
                            spb[:, 512:1024],
                            k_rot[:, h, k1sl],
                            q_rot[:, h, q_lo : q_lo + 512],
                            start=True,
                            stop=True,
                        )
                        nc.scalar.activation(pp[:, :, :], spb[:, 0:1024], EXP, scale=esc)
                        return 0
                    q0 = 128 * (ki0 - 4 * qc)
                    q0b = q0 + 128
                    nc.tensor.matmul(
                        spb[:, q0:512],
                        k_rot[:, h, k0sl],
                        q_rot[:, h, q_lo + q0 : q_lo + 512],
                        start=True,
                        stop=True,
                    )
                    nc.tensor.matmul(
                        spb[:, 512 + q0b : 1024],
                        k_rot[:, h, k1sl],
                        q_rot[:, h, q_lo + q0b : q_lo + 512],
                        start=True,
                        stop=True,
                    )
                    nc.scalar.activation(
                        pp[:, 0, q0:512], spb[:, q0:512], EXP, scale=esc
                    )
                    nc.scalar.activation(
                        pp[:, 1, q0b:512], spb[:, 512 + q0b : 1024], EXP, scale=esc
                    )
                    nc.vector.memset(pp[:, 1, q0:q0b], 0.0)
                    nc.vector.tensor_mul(
                        pp[:, 0, q0 : q0 + 128],
                        pp[:, 0, q0 : q0 + 128],
                        maskd8[:, k0sl],
                    )
                    nc.vector.tensor_mul(
                        pp[:, 1, q0b : q0b + 128],
                        pp[:, 1, q0b : q0b + 128],
                        maskd8[:, k1sl],
                    )
                    return q0

                def sums_av_pair(h, kp, npair, pp, q0, sums, avp):
                    hsl = slice(h * 128, (h + 1) * 128)
                    nc.tensor.matmul(
                        sums[:, q0:512],
                        ones8[:, :, :],
                        pp[:, :, q0:512],
                        start=(kp == 0),
                        stop=(kp == npair - 1),
                        perf_mode=DR,
                    )
                    nc.tensor.matmul(
                        avp[:, q0:512],
                        v8[:, 2 * kp : 2 * kp + 2, hsl],
                        pp[:, :, q0:512],
                        start=(kp == 0),
                        stop=(kp == npair - 1),
                        perf_mode=DR,
                    )

                def finish_head(qc, h, sums, avp):
                    rc = rcp.tile([128, 512], F32, tag="rc")
                    nc.vector.reciprocal_approx_fast(rc[:, :], sums[:, :])
                    if qc == 0:
                        nc.vector.tensor_mul(
                            aoT16[:, h, :], avp[:, 0:CL], rc[:, 0:CL]
                        )
                        nc.vector.tensor_mul(
                            aoT8[:, h, CL:512], avp[:, CL:512], rc[:, CL:512]
                        )
                    else:
                        q_lo = qc * 512
                        nc.vector.tensor_mul(
                            aoT8[:, h, q_lo : q_lo + 512], avp[:, :], rc[:, :]
                        )

                def attn_head0_clean(h):
                    """qc=0: rows 0:128 bf16 via ki0; rows 128:512 fp8 pairs."""
                    hsl = slice(h * 128, (h + 1) * 128)
                    sav = accp.tile([128, 1024], F32, tag="acc")
                    sums, avp = sav[:, 0:512], sav[:, 512:1024]
                    # pair 0 (ki0 full block, ki1 diag at q0=128)
                    spb = spp.tile([128, 1024], F32, tag="sp")
                    pp = pp8p.tile([128, 2, 512], FP8, tag="pp8")
                    ppc = ppcp.tile([128, CL], BF16, tag="ppc")
                    nc.tensor.matmul(
                        spb[:, 0:512],
                        k_rot[:, h, 0:128],
                        q_rot[:, h, 0:512],
                        start=True,
                        stop=True,
                    )
                    nc.tensor.matmul(
                        spb[:, 512 + 128 : 1024],
                        k_rot[:, h, 128:256],
                        q_rot[:, h, 128:512],
                        start=True,
                        stop=True,
                    )
                    # fp8 exps cover q 128:512; bf16 exp covers q 0:128
                    nc.scalar.activation(pp[:, 0, 128:512], spb[:, 128:512], EXP, scale=esc)
                    nc.scalar.activation(
                        pp[:, 1, 128:512], spb[:, 640:1024], EXP, scale=esc
                    )
                    nc.scalar.activation(ppc[:, :], spb[:, 0:CL], EXP, scale=esc)
                    nc.vector.tensor_mul(
                        pp[:, 1, 128:256], pp[:, 1, 128:256], maskd8[:, 128:256]
                    )
                    nc.vector.tensor_mul(ppc[:, :], ppc[:, :], maskd16[:, :])
                    nc.tensor.matmul(
                        sums[:, 128:512],
                        ones8[:, :, :],
                        pp[:, :, 128:512],
                        start=True,
                        stop=False,
                        perf_mode=DR,
                    )
                    nc.tensor.matmul(
                        avp[:, 128:512],
                        v8[:, 0:2, hsl],
                        pp[:, :, 128:512],
                        start=True,
                        stop=False,
                        perf_mode=DR,
                    )
                    # same PSUM banks as the fp8 pair above: accumulate-only
                    # (the bank's start/stop live on the fp8 pair matmuls)
                    nc.tensor.matmul(
                        sums[:, 0:CL], ones16[:, :], ppc[:, :], start=False, stop=False
                    )
                    nc.tensor.matmul(
                        avp[:, 0:CL], v16[:, 0, hsl], ppc[:, :], start=False, stop=False
                    )
                    # pair 1 (ki2/ki3 diag at q0=256/384)
                    pp = pp8p.tile([128, 2, 512], FP8, tag="pp8")
                    q0 = scores_pair(0, h, 1, pp)
                    nc.tensor.matmul(
                        sums[:, q0:512],
                        ones8[:, :, :],
                        pp[:, :, q0:512],
                        start=False,
                        stop=True,
                        perf_mode=DR,
                    )
                    nc.tensor.matmul(
                        avp[:, q0:512],
                        v8[:, 2:4, hsl],
                        pp[:, :, q0:512],
                        start=False,
                        stop=True,
                        perf_mode=DR,
                    )
                    finish_head(0, h, sums, avp)

                def attn_head(qc, h, peeled=None):
                    sav = accp.tile([128, 1024], F32, tag="acc")
                    sums, avp = sav[:, 0:512], sav[:, 512:1024]
                    npair = 2 * qc + 2
                    for kp in range(npair):
                        if peeled is not None and kp < len(peeled):
                            pp, q0 = peeled[kp]
                        else:
                            pp = pp8p.tile([128, 2, 512], FP8, tag="pp8")
                            q0 = scores_pair(qc, h, kp, pp)
                        sums_av_pair(h, kp, npair, pp, q0, sums, avp)
                    finish_head(qc, h, sums, avp)

                def outproj(qc):
                    for sb in range(4 * qc, 4 * qc + 4):
                        ssl = slice(sb * 128, (sb + 1) * 128)
                        for oc2 in range(2):
                            op2 = accp.tile([128, 1024], F32, tag="acc")
                            o0 = slice((2 * oc2) * 512, (2 * oc2 + 1) * 512)
                            o1 = slice((2 * oc2 + 1) * 512, (2 * oc2 + 2) * 512)
                            if sb == 0:
                                for h in range(HPC):
                                    lhsT = aoT16[:, h, :]
                                    nc.tensor.matmul(
                                        op2[:, 0:512],
                                        lhsT,
                                        wo8[:, h, o0],
                                        start=(h == 0),
                                        stop=(h == HPC - 1),
                                    )
                                    nc.tensor.matmul(
                                        op2[:, 512:1024],
                                        lhsT,
                                        wo8[:, h, o1],
                                        start=(h == 0),
                                        stop=(h == HPC - 1),
                                    )
                            else:
                                for hp in range(2):
                                    hpsl = slice(2 * hp, 2 * hp + 2)
                                    lhsT = aoT8[:, hpsl, ssl]
                                    nc.tensor.matmul(
                                        op2[:, 0:512],
                                        lhsT,
                                        wo8[:, hpsl, o0],
                                        start=(hp == 0),
                                        stop=(hp == 1),
                                        perf_mode=DR,
                                    )
                                    nc.tensor.matmul(
                                        op2[:, 512:1024],
                                        lhsT,
                                        wo8[:, hpsl, o1],
                                        start=(hp == 0),
                                        stop=(hp == 1),
                                        perf_mode=DR,
                                    )
                            ob = osbp.tile([128, 1024], BF16, tag="osb")
                            if evict_flip[0] % 2 == 0:
                                nc.scalar.copy(ob[:, :], op2[:, :])
                                nc.scalar.dma_start(
                                    out=out_d[ssl, oc2 * 1024 : (oc2 + 1) * 1024],
                                    in_=ob[:, :],
                                )
                            else:
                                nc.vector.tensor_copy(ob[:, :], op2[:, :])
                                nc.sync.dma_start(
                                    out=out_d[ssl, oc2 * 1024 : (oc2 + 1) * 1024],
                                    in_=ob[:, :],
                                )
                            evict_flip[0] += 1

                for h in range(HPC):
                    attn_head0_clean(h)
                for qc in range(1, NQC):
                    # peel the next chunk's first scores so the PE has work
                    # while the previous chunk's aoT dependency chain drains.
                    peeled = []
                    for kp in range(2):
                        pp = pp8p.tile([128, 2, 512], FP8, tag="pp8")
                        q0 = scores_pair(qc, 0, kp, pp)
                        peeled.append((pp, q0))
                    outproj(qc - 1)
                    attn_head(qc, 0, peeled=peeled)
                    for h in range(1, HPC):
                        attn_head(qc, h)
                outproj(NQC - 1)

    nc.finalize()
    return nc


def _bit_quantize_ternary(w: np.ndarray):
    """Returns (ternary {-1,0,1} float32 matrix, scale) matching the reference."""
    scale = np.maximum(np.mean(np.abs(w.astype(np.float32))), np.float32(1e-5))
    t = np.clip(np.round(w.astype(np.float32) / scale), -1.0, 1.0).astype(np.float32)
    return t, float(scale)


def _host_tables():
    """cos/sin stacked [128, S]: rows 0:64 cos, rows 64:128 sin."""
    inv_freq = 1.0 / (ROPE_BASE ** (np.arange(0, D_HEAD, 2, dtype=np.float32) / D_HEAD))
    pos = np.arange(SEQ, dtype=np.float32)
    ang = pos[:, None] * inv_freq[None, :]  # [S, 64]
    cs = np.empty((128, SEQ), dtype=NPBF16)
    cs[0:64] = np.ascontiguousarray(np.cos(ang).T).astype(NPBF16)
    cs[64:128] = np.ascontiguousarray(np.sin(ang).T).astype(NPBF16)
    sc = np.empty((128, SEQ), dtype=NPBF16)
    sc[0:64] = cs[64:128]
    sc[64:128] = cs[0:64]
    return cs, sc


def kernel(x, w_qkv, w_out, mask):
    global LAST_RESULT
    x = np.asarray(x, dtype=np.float32)
    w_qkv = np.asarray(w_qkv, dtype=np.float32)
    w_out = np.asarray(w_out, dtype=np.float32)
    mask = np.asarray(mask)

    tq, sq = _bit_quantize_ternary(w_qkv)
    to, so = _bit_quantize_ternary(w_out)
    exp_scale = (sq * sq) / float(np.sqrt(D_HEAD))
    # wv is pre-scaled by 1/4 on upload (fp8 range); compensate here.
    c2 = np.float32(sq * so * 4.0)

    m2 = (mask.reshape(SEQ, SEQ) != 0).astype(np.float32)
    causal = bool(np.array_equal(m2, np.tril(np.ones((SEQ, SEQ), np.float32))))
    assert causal, "kernel specialized for the causal mask"

    cs, sc = _host_tables()
    maskd8 = np.empty((128, SEQ), dtype=NPFP8)
    for ki in range(NKI):
        blk = m2[ki * 128 : (ki + 1) * 128, ki * 128 : (ki + 1) * 128]  # [q, k]
        maskd8[:, ki * 128 : (ki + 1) * 128] = np.ascontiguousarray(blk.T).astype(
            NPFP8
        )
    maskd16 = maskd8[:, 0:CL].astype(NPBF16)

    key = float(exp_scale)
    if key not in _PROG_CACHE:
        _PROG_CACHE[key] = _build_program(float(exp_scale))
    nc = _PROG_CACHE[key]

    in_maps = []
    for c in range(N_CORES):
        b, g = divmod(c, 4)
        rows = slice(R_LOCAL * g, R_LOCAL * (g + 1))
        xT = np.ascontiguousarray(x[b].T)
        im = {
            "xT8": xT.astype(NPFP8),
            "xT16": np.ascontiguousarray(xT[:, 0:CL]).astype(NPBF16),
            "wqT": np.ascontiguousarray(tq[0 * D_MODEL :][rows].T).astype(NPFP8),
            "wkT": np.ascontiguousarray(tq[1 * D_MODEL :][rows].T).astype(NPFP8),
            "wvT": np.ascontiguousarray(tq[2 * D_MODEL :][rows].T * 0.25).astype(
                NPFP8
            ),
            "woT": np.ascontiguousarray(to[:, rows].T).astype(NPFP8),
            "cossinT": cs,
            "sincosT": sc,
            "maskd16": maskd16,
            "maskd8": maskd8,
        }
        in_maps.append(im)

    do_trace = bool(PROFILE) and _enable_profiling()
    res = run_bass_kernel_spmd(nc, in_maps, list(range(N_CORES)), trace=do_trace)
    LAST_RESULT = res

    parts = [np.asarray(res.results[c]["out"]).astype(np.float32) for c in range(N_CORES)]
    out = np.stack(
        [
            parts[0] + parts[1] + parts[2] + parts[3],
            parts[4] + parts[5] + parts[6] + parts[7],
        ]
    )
    return (out * c2).astype(np.float32)


# revision 15
# speedup vs baseline: 1.0728x; 1.0728x over previous
"""Trainium2 Bass kernel for BitNet multi-head attention (nn_MultiHeadAttention_62294205661880).

Sharding: 8 cores = 2 batches x 4 head-groups (4 heads each).  Each core
computes qkv projection, RoPE, causal attention and a column-parallel slice
of the output projection for its (batch, head-group); the host sums the 4
partial out-projections per batch.

v3: fp8 DoubleRow everywhere except the first 128 queries.  fp8 quantization
error concentrates in the earliest rows (few keys -> no averaging), so rows
0:128 take a bf16 "clean" path while everything else runs fp8 DoubleRow
matmuls (2 contraction blocks per PE pass):
  - Q/K projection: output cols 0:128 from bf16 x, cols 128:2048 DoubleRow
    from fp8 x.
  - V projection: key block 0 from bf16 x (stored bf16+fp8), blocks 1:15
    DoubleRow (stored fp8).  wv is pre-scaled by 1/4 so v fits fp8 range
    (max |v| = 240.5 > 240 = trn-e4m3 max).
  - attention: softmax numerator exp() written as fp8 ki-pairs; denominator
    (ones-matmul) and AV both DoubleRow over ki-pairs.  Rows 0:128 of query
    chunk 0 keep a bf16 sub-path (scores block ki0 -> bf16 exp/sums/AV).
  - out-projection: attention output stored fp8, head-pair DoubleRow
    (seq block 0 bf16 from the clean attention output).
Weights are ternary {-1,0,+1} (x0.25 for wv) -- exact in fp8, so DoubleRow
matmuls only add the activation-side quantization noise.  Simulated
rel-err 0.0045 vs the 2e-2 gate.

Input DMAs are split across both hardware DGE queues (sync + scalar
engines) so the projection matmuls are fed at ~2x single-queue bandwidth;
out-projection emission is software-pipelined behind the next chunk's
first score matmuls to hide the aoT dependency chain.

Device layout trick (unchanged): everything is computed transposed.  Q_T/K_T
come out of the projection as [dh, S]; scores are s_T[k, q]; the softmax
denominator sums over the partition (key) dim via an all-ones stationary
matmul; AV produces out_T[dh, q] which feeds the output projection directly.
No on-device transposes.  Softmax skips the max-subtraction: scores are
bounded (~+-2) because the BitNet weight scale is tiny.
"""

import sys
import types

import numpy as np
import ml_dtypes

import concourse.bass as bass
import concourse.mybir as mybir
import concourse.tile as tile
from concourse import bacc
from concourse.bass_utils import run_bass_kernel_spmd

D_MODEL = 2048
N_HEADS = 16
D_HEAD = 128
SEQ = 2048
BATCH = 2
ROPE_BASE = 10000.0

N_CORES = 8
HPC = 4  # heads per core
R_LOCAL = HPC * D_HEAD  # 512 local q (or k, or v) rows per core
MO = D_MODEL // 128  # 16 contraction blocks
MO2 = MO // 2  # 8 contraction block pairs
NKI = SEQ // 128  # 16 key blocks
NQC = SEQ // 512  # 4 query chunks of 512
NSB = SEQ // 128  # 16 seq blocks (v / proj)
CL = 128  # clean (bf16-path) rows

BF16 = mybir.dt.bfloat16
F32 = mybir.dt.float32
NPBF16 = ml_dtypes.bfloat16
NPFP8 = ml_dtypes.float8_e4m3
FP8 = mybir.dt.float8e4
DR = mybir.MatmulPerfMode.DoubleRow

LAST_RESULT = None  # BassKernelResults of the most recent run (for test.py)
_PROG_CACHE = {}
PROFILE = False  # test.py sets True to capture an NTFF profile / HW exec time


def _enable_profiling() -> bool:
    """Install the axon NTFF profile hook glue if the image lacks
    ``antenv.axon_hooks`` (boot degrades silently without it), and skip
    the artifact upload (no bucket access in this container)."""
    try:
        from antenv.axon_hooks import get_axon_ntff_profile_hook  # noqa: F401

        ok = get_axon_ntff_profile_hook() is not None
    except ImportError:
        ok = False
        import antenv

        mod = types.ModuleType("antenv.axon_hooks")
        mod._hook = None
        mod.set_axon_ntff_profile_hook = lambda h: setattr(mod, "_hook", h)
        mod.get_axon_ntff_profile_hook = lambda: mod._hook
        sys.modules["antenv.axon_hooks"] = mod
        antenv.axon_hooks = mod
        try:
            from trn_agent_boot.trn_boot import _ntff_profile_via_ctypes

            hook = _ntff_profile_via_ctypes("/opt/axon/libaxon_pjrt.so")
            if hook is not None:
                mod._hook = hook
                ok = True
        except Exception as e:  # profiling is best-effort
            print(f"ntff profile hook install failed: {e}", file=sys.stderr)
    if ok:
        import concourse.bass_utils as _bu

        _bu.upload_artifacts = lambda tmpdir: tmpdir
    return ok


def _build_program(exp_scale: float) -> bass.Bass:
    nc = bacc.Bacc(None)
    S = SEQ

    xT8_d = nc.dram_tensor("xT8", [D_MODEL, S], FP8, kind="ExternalInput")
    xT16_d = nc.dram_tensor("xT16", [D_MODEL, CL], BF16, kind="ExternalInput")
    wqT_d = nc.dram_tensor("wqT", [D_MODEL, R_LOCAL], FP8, kind="ExternalInput")
    wkT_d = nc.dram_tensor("wkT", [D_MODEL, R_LOCAL], FP8, kind="ExternalInput")
    wvT_d = nc.dram_tensor("wvT", [D_MODEL, R_LOCAL], FP8, kind="ExternalInput")
    woT_d = nc.dram_tensor("woT", [R_LOCAL, D_MODEL], FP8, kind="ExternalInput")
    # cos rows 0:64, sin rows 64:128
    cs_d = nc.dram_tensor("cossinT", [128, S], BF16, kind="ExternalInput")
    # swapped: sin rows 0:64, cos rows 64:128 (keeps TensorTensor base partitions equal)
    sc_d = nc.dram_tensor("sincosT", [128, S], BF16, kind="ExternalInput")
    # transposed diagonal 128x128 mask blocks, side by side (bf16: block 0 only)
    maskd16_d = nc.dram_tensor("maskd16", [128, CL], BF16, kind="ExternalInput")
    maskd8_d = nc.dram_tensor("maskd8", [128, S], FP8, kind="ExternalInput")
    out_d = nc.dram_tensor("out", [S, D_MODEL], BF16, kind="ExternalOutput")

    xT8_v = xT8_d[:].rearrange("(mo p) s -> p mo s", p=128)
    xT16_v = xT16_d[:].rearrange("(mo p) s -> p mo s", p=128)
    wqT_v = wqT_d[:].rearrange("(mo p) r -> p mo r", p=128)
    wkT_v = wkT_d[:].rearrange("(mo p) r -> p mo r", p=128)
    wvT_v = wvT_d[:].rearrange("(mo p) r -> p mo r", p=128)
    woT_v = woT_d[:].rearrange("(h p) o -> p h o", p=128)

    with tile.TileContext(nc) as tc:
        with tc.tile_pool(name="pers", bufs=1) as pers:
            # ---- persistent SBUF tensors (live across both phases) ----
            q_rot = pers.tile([128, HPC, S], BF16, tag="qrot")
            k_rot = pers.tile([128, HPC, S], BF16, tag="krot")
            v8 = pers.tile([128, NKI, R_LOCAL], FP8, tag="v8")
            v16 = pers.tile([128, 1, R_LOCAL], BF16, tag="v16")
            aoT8 = pers.tile([128, HPC, S], FP8, tag="aoT8")
            aoT16 = pers.tile([128, HPC, CL], BF16, tag="aoT16")
            wo8 = pers.tile([128, HPC, D_MODEL], FP8, tag="wo8")
            ones16 = pers.tile([128, 128], BF16, tag="ones16")
            ones8 = pers.tile([128, 2, 128], FP8, tag="ones8")
            warm = pers.tile([128, 1], BF16, tag="warm")
            maskd16 = pers.tile([128, CL], BF16, tag="maskd16")
            maskd8 = pers.tile([128, S], FP8, tag="maskd8")
            nc.vector.memset(ones16[:, :], 1.0)
            nc.vector.memset(ones8[:, :, :], 1.0)
            # load the exp table set first so no ACT table switch happens
            # mid-kernel (Copy lives in every set).
            nc.scalar.activation(
                warm[:, :], ones16[:, 0:1], mybir.ActivationFunctionType.Exp
            )

            # ================= phase A: QKV projection + RoPE =================
            with (
                tc.tile_pool(name="xtp", bufs=1) as xtp,
                tc.tile_pool(name="wp", bufs=1) as wp,
                tc.tile_pool(name="raw", bufs=3) as rawp,
                tc.tile_pool(name="tmp", bufs=2) as tmpp,
                tc.tile_pool(name="psA", bufs=2, space="PSUM") as psA,
            ):
                xt8 = xtp.tile([128, MO, S], FP8, tag="xt8")  # cols 0:128 unused
                xt16 = xtp.tile([128, MO, CL], BF16, tag="xt16")
                wq = wp.tile([128, MO, R_LOCAL], FP8, tag="wq")
                wk = wp.tile([128, MO, R_LOCAL], FP8, tag="wk")
                wv = wp.tile([128, MO, R_LOCAL], FP8, tag="wv")
                cs_t = wp.tile([128, S], BF16, tag="cs")
                sc_t = wp.tile([128, S], BF16, tag="sc")

                # Input DMAs on both hardware DGE queues (~195 GB/s each).
                # Head 0 runs its bf16 clean m-loop first (small xt16, lands
                # early on queue B) while the xt8 pairs stream on both queues
                # interleaved with wq/wk, so the DR m2-outer loop never
                # starves.  queue B (scalar): xt16, even xt8 pairs, cs/sc,
                # wo8.  queue A (sync): wq/wk, odd xt8 pairs, wv, masks.
                def dma_x8(eng, m2):
                    sl = slice(2 * m2, 2 * m2 + 2)
                    if m2 == 0:
                        for c4 in range(4):
                            lo = 128 if c4 == 0 else c4 * 512
                            eng.dma_start(
                                out=xt8[:, sl, lo : (c4 + 1) * 512],
                                in_=xT8_v[:, sl, lo : (c4 + 1) * 512],
                            )
                    else:
                        eng.dma_start(out=xt8[:, sl, 128:S], in_=xT8_v[:, sl, 128:S])

                nc.sync.dma_start(out=wq[:, :, :], in_=wqT_v[:, :, :])
                nc.scalar.dma_start(out=xt16[:, :, :], in_=xT16_v[:, :, :])
                nc.scalar.dma_start(out=wk[:, :, :], in_=wkT_v[:, :, :])
                dma_x8(nc.sync, 0)
                dma_x8(nc.sync, 1)
                dma_x8(nc.scalar, 2)
                dma_x8(nc.sync, 3)
                dma_x8(nc.scalar, 4)
                dma_x8(nc.sync, 5)
                dma_x8(nc.scalar, 6)
                dma_x8(nc.sync, 7)
                nc.scalar.dma_start(out=cs_t[:, :], in_=cs_d[:, :])
                nc.scalar.dma_start(out=sc_t[:, :], in_=sc_d[:, :])
                nc.sync.dma_start(out=wv[:, :, :], in_=wvT_v[:, :, :])
                nc.scalar.dma_start(out=wo8[:, :, :], in_=woT_v[:, :, :])
                nc.sync.dma_start(out=maskd16[:, :], in_=maskd16_d[:, :])
                nc.sync.dma_start(out=maskd8[:, :], in_=maskd8_d[:, :])

                def rope(dst, raw):
                    """NeoX rotary: rows 0:64 = t*c - b*s ; rows 64:128 = t*s + b*c."""
                    ta = tmpp.tile([64, S], BF16, tag="tmp")
                    tb = tmpp.tile([64, S], BF16, tag="tmp")
                    nc.vector.tensor_mul(ta[:, :], raw[0:64, :], cs_t[0:64, :])
                    nc.vector.tensor_mul(tb[:, :], raw[64:128, :], cs_t[64:128, :])
                    nc.vector.tensor_sub(dst[0:64, :], ta[:, :], tb[:, :])
                    tc2 = tmpp.tile([64, S], BF16, tag="tmp")
                    td = tmpp.tile([64, S], BF16, tag="tmp")
                    nc.vector.tensor_mul(tc2[:, :], raw[0:64, :], sc_t[0:64, :])
                    nc.vector.tensor_mul(td[:, :], raw[64:128, :], sc_t[64:128, :])
                    nc.vector.tensor_add(dst[64:128, :], tc2[:, :], td[:, :])

                def dr_chunks(ps, w_sb, hsl, msl, first, last):
                    """DoubleRow q/k chunk matmuls for one m-pair (cols 128:2048).

                    The c4=0 chunk shares its PSUM bank (2KB zero region) with
                    the bf16 clean cols 0:128, whose matmuls come later: only
                    the first matmul per bank may carry start=True, and the
                    bank's stop stays with its last writer (the clean loop)."""
                    for c4 in range(4):
                        lo = 128 if c4 == 0 else c4 * 512
                        csl = slice(lo, (c4 + 1) * 512)
                        nc.tensor.matmul(
                            ps[:, csl],
                            w_sb[:, msl, hsl],
                            xt8[:, msl, csl],
                            start=first,
                            stop=last and c4 != 0,
                            perf_mode=DR,
                        )

                # head 0 q/k: bf16 clean m-loop first (xt16 lands first),
                # then the DR m2-outer loop consumes xt8 m-pairs as the DMAs
                # land (startup overlap).  The clean loop owns zone0's
                # start=True; the c4=0 DR chunk carries its stop.
                qp0 = psA.tile([128, S], F32, tag="psA")
                kp0 = psA.tile([128, S], F32, tag="psA")
                for ps, w_sb in ((qp0, wq), (kp0, wk)):
                    for m in range(MO):
                        nc.tensor.matmul(
                            ps[:, 0:CL],
                            w_sb[:, m, 0:128],
                            xt16[:, m, :],
                            start=(m == 0),
                            stop=False,
                        )
                for m2 in range(MO2):
                    msl = slice(2 * m2, 2 * m2 + 2)
                    for ps, w_sb in ((qp0, wq), (kp0, wk)):
                        for c4 in range(4):
                            lo = 128 if c4 == 0 else c4 * 512
                            nc.tensor.matmul(
                                ps[:, lo : (c4 + 1) * 512],
                                w_sb[:, msl, 0:128],
                                xt8[:, msl, lo : (c4 + 1) * 512],
                                start=(m2 == 0 and c4 != 0),
                                stop=(m2 == MO2 - 1),
                                perf_mode=DR,
                            )
                q_raw = rawp.tile([128, S], BF16, tag="raw")
                nc.scalar.copy(q_raw[:, :], qp0[:, :])
                rope(q_rot[:, 0, :], q_raw)
                k_raw = rawp.tile([128, S], BF16, tag="raw")
                nc.scalar.copy(k_raw[:, :], kp0[:, :])
                rope(k_rot[:, 0, :], k_raw)

                def project(dst_raw, w_sb, h):
                    """q/k head projection -> bf16 raw [128, S] (xt resident)."""
                    hsl = slice(h * 128, (h + 1) * 128)
                    ps = psA.tile([128, S], F32, tag="psA")
                    for m2 in range(MO2):
                        msl = slice(2 * m2, 2 * m2 + 2)
                        dr_chunks(ps, w_sb, hsl, msl, m2 == 0, m2 == MO2 - 1)
                    for m in range(MO):
                        nc.tensor.matmul(
                            ps[:, 0:CL],
                            w_sb[:, m, hsl],
                            xt16[:, m, :],
                            start=False,
                            stop=(m == MO - 1),
                        )
                    nc.scalar.copy(dst_raw[:, :], ps[:, :])

                def v_group(sb4):
                    """V projection group (4 seq blocks, natural layout [s, r]).
                    sb0 (keys 0:128) from bf16 x, stored bf16 + fp8; the rest
                    DoubleRow from fp8 x, stored fp8 only."""
                    ps = psA.tile([128, S], F32, tag="psA")
                    for part in range(4):
                        sb = sb4 * 4 + part
                        ssl = slice(sb * 128, (sb + 1) * 128)
                        psl = slice(part * 512, part * 512 + 512)
                        if sb == 0:
                            for m in range(MO):
                                nc.tensor.matmul(
                                    ps[:, psl],
                                    xt16[:, m, :],
                                    wv[:, m, :],
                                    start=(m == 0),
                                    stop=(m == MO - 1),
                                )
                        else:
                            for m2 in range(MO2):
                                msl = slice(2 * m2, 2 * m2 + 2)
                                nc.tensor.matmul(
                                    ps[:, psl],
                                    xt8[:, msl, ssl],
                                    wv[:, msl, :],
                                    start=(m2 == 0),
                                    stop=(m2 == MO2 - 1),
                                    perf_mode=DR,
                                )
                    # evictions all on ACT: DVE is saturated by RoPE and any
                    # DVE eviction here stalls PSUM reuse behind the rope
                    # backlog.
                    k0 = sb4 * 4
                    if sb4 == 0:
                        nc.scalar.copy(v16[:, 0, :], ps[:, 0:512])
                        nc.scalar.copy(v8[:, 0:2, :], ps[:, 0:1024])
                        nc.scalar.copy(v8[:, 2:4, :], ps[:, 1024:2048])
                    else:
                        nc.scalar.copy(v8[:, k0 : k0 + 2, :], ps[:, 0:1024])
                        nc.scalar.copy(v8[:, k0 + 2 : k0 + 4, :], ps[:, 1024:2048])

                for h in range(1, HPC):
                    q_raw = rawp.tile([128, S], BF16, tag="raw")
                    project(q_raw, wq, h)
                    rope(q_rot[:, h, :], q_raw)
                    k_raw = rawp.tile([128, S], BF16, tag="raw")
                    project(k_raw, wk, h)
                    rope(k_rot[:, h, :], k_raw)
                for sb4 in range(NSB // 4):
                    v_group(sb4)

            # ================= phase B: attention + out-projection =============
            with (
                tc.tile_pool(name="pp8", bufs=6) as pp8p,
                tc.tile_pool(name="ppc", bufs=2) as ppcp,
                tc.tile_pool(name="rcp", bufs=3) as rcp,
                tc.tile_pool(name="osb", bufs=4) as osbp,
                tc.tile_pool(name="sp", bufs=2, space="PSUM") as spp,
                tc.tile_pool(name="acc", bufs=2, space="PSUM") as accp,
            ):
                EXP = mybir.ActivationFunctionType.Exp
                esc = float(exp_scale)
                evict_flip = [0]

                def scores_pair(qc, h, kp, pp):
                    """scores + exp + mask for ki-pair kp -> pp [128, 2, 512].
                    Returns q0, the first valid column of the pair."""
                    q_lo = qc * 512
                    ki0, ki1 = 2 * kp, 2 * kp + 1
                    k0sl = slice(ki0 * 128, (ki0 + 1) * 128)
                    k1sl = slice(ki1 * 128, (ki1 + 1) * 128)
                    spb = spp.tile([128, 1024], F32, tag="sp")
                    if kp < 2 * qc:  # off-diagonal pair
                        nc.tensor.matmul(
                            spb[:, 0:512],
                            k_rot[:, h, k0sl],
                            q_rot[:, h, q_lo : q_lo + 512],
                            start=True,
                            stop=True,
                        )
                        nc.tensor.matmul(
                            spb[:, 512:1024],
                            k_rot[:, h, k1sl],
                            q_rot[:, h, q_lo : q_lo + 512],
                            start=True,
                            stop=True,
                        )
                        nc.scalar.activation(pp[:, :, :], spb[:, 0:1024], EXP, scale=esc)
                        return 0
                    q0 = 128 * (ki0 - 4 * qc)
                    q0b = q0 + 128
                    nc.tensor.matmul(
                        spb[:, q0:512],
                        k_rot[:, h, k0sl],
                        q_rot[:, h, q_lo + q0 : q_lo + 512],
                        start=True,
                        stop=True,
                    )
                    nc.tensor.matmul(
                        spb[:, 512 + q0b : 1024],
                        k_rot[:, h, k1sl],
                        q_rot[:, h, q_lo + q0b : q_lo + 512],
                        start=True,
                        stop=True,
                    )
                    nc.scalar.activation(
                        pp[:, 0, q0:512], spb[:, q0:512], EXP, scale=esc
                    )
                    nc.scalar.activation(
                        pp[:, 1, q0b:512], spb[:, 512 + q0b : 1024], EXP, scale=esc
                    )
                    nc.vector.memset(pp[:, 1, q0:q0b], 0.0)
                    nc.vector.tensor_mul(
                        pp[:, 0, q0 : q0 + 128],
                        pp[:, 0, q0 : q0 + 128],
                        maskd8[:, k0sl],
                    )
                    nc.vector.tensor_mul(
                        pp[:, 1, q0b : q0b + 128],
                        pp[:, 1, q0b : q0b + 128],
                        maskd8[:, k1sl],
                    )
                    return q0

                def sums_av_pair(h, kp, npair, pp, q0, sums, avp):
                    hsl = slice(h * 128, (h + 1) * 128)
                    nc.tensor.matmul(
                        sums[:, q0:512],
                        ones8[:, :, :],
                        pp[:, :, q0:512],
                        start=(kp == 0),
                        stop=(kp == npair - 1),
                        perf_mode=DR,
                    )
                    nc.tensor.matmul(
                        avp[:, q0:512],
                        v8[:, 2 * kp : 2 * kp + 2, hsl],
                        pp[:, :, q0:512],
                        start=(kp == 0),
                        stop=(kp == npair - 1),
                        perf_mode=DR,
                    )

                def finish_head(qc, h, sums, avp):
                    rc = rcp.tile([128, 512], F32, tag="rc")
                    nc.vector.reciprocal_approx_fast(rc[:, :], sums[:, :])
                    if qc == 0:
                        nc.vector.tensor_mul(
                            aoT16[:, h, :], avp[:, 0:CL], rc[:, 0:CL]
                        )
                        nc.vector.tensor_mul(
                            aoT8[:, h, CL:512], avp[:, CL:512], rc[:, CL:512]
                        )
                    else:
                        q_lo = qc * 512
                        nc.vector.tensor_mul(
                            aoT8[:, h, q_lo : q_lo + 512], avp[:, :], rc[:, :]
                        )

                def attn_head0_clean(h):
                    """qc=0: rows 0:128 bf16 via ki0; rows 128:512 fp8 pairs."""
                    hsl = slice(h * 128, (h + 1) * 128)
                    sav = accp.tile([128, 1024], F32, tag="acc")
                    sums, avp = sav[:, 0:512], sav[:, 512:1024]
                    # pair 0 (ki0 full block, ki1 diag at q0=128)
                    spb = spp.tile([128, 1024], F32, tag="sp")
                    pp = pp8p.tile([128, 2, 512], FP8, tag="pp8")
                    ppc = ppcp.tile([128, CL], BF16, tag="ppc")
                    nc.tensor.matmul(
                        spb[:, 0:512],
                        k_rot[:, h, 0:128],
                        q_rot[:, h, 0:512],
                        start=True,
                        stop=True,
                    )
                    nc.tensor.matmul(
                        spb[:, 512 + 128 : 1024],
                        k_rot[:, h, 128:256],
                        q_rot[:, h, 128:512],
                        start=True,
                        stop=True,
                    )
                    # fp8 exps cover q 128:512; bf16 exp covers q 0:128
                    nc.scalar.activation(pp[:, 0, 128:512], spb[:, 128:512], EXP, scale=esc)
                    nc.scalar.activation(
                        pp[:, 1, 128:512], spb[:, 640:1024], EXP, scale=esc
                    )
                    nc.scalar.activation(ppc[:, :], spb[:, 0:CL], EXP, scale=esc)
                    nc.vector.tensor_mul(
                        pp[:, 1, 128:256], pp[:, 1, 128:256], maskd8[:, 128:256]
                    )
                    nc.vector.tensor_mul(ppc[:, :], ppc[:, :], maskd16[:, :])
                    nc.tensor.matmul(
                        sums[:, 128:512],
                        ones8[:, :, :],
                        pp[:, :, 128:512],
                        start=True,
                        stop=False,
                        perf_mode=DR,
                    )
                    nc.tensor.matmul(
                        avp[:, 128:512],
                        v8[:, 0:2, hsl],
                        pp[:, :, 128:512],
                        start=True,
                        stop=False,
                        perf_mode=DR,
                    )
                    # same PSUM banks as the fp8 pair above: accumulate-only
                    # (the bank's start/stop live on the fp8 pair matmuls)
                    nc.tensor.matmul(
                        sums[:, 0:CL], ones16[:, :], ppc[:, :], start=False, stop=False
                    )
                    nc.tensor.matmul(
                        avp[:, 0:CL], v16[:, 0, hsl], ppc[:, :], start=False, stop=False
                    )
                    # pair 1 (ki2/ki3 diag at q0=256/384)
                    pp = pp8p.tile([128, 2, 512], FP8, tag="pp8")
                    q0 = scores_pair(0, h, 1, pp)
                    nc.tensor.matmul(
                        sums[:, q0:512],
                        ones8[:, :, :],
                        pp[:, :, q0:512],
                        start=False,
                        stop=True,
                        perf_mode=DR,
                    )
                    nc.tensor.matmul(
                        avp[:, q0:512],
                        v8[:, 2:4, hsl],
                        pp[:, :, q0:512],
                        start=False,
                        stop=True,
                        perf_mode=DR,
                    )
                    finish_head(0, h, sums, avp)

                def attn_head(qc, h, peeled=None):
                    sav = accp.tile([128, 1024], F32, tag="acc")
                    sums, avp = sav[:, 0:512], sav[:, 512:1024]
                    npair = 2 * qc + 2
                    for kp in range(npair):
                        if peeled is not None and kp < len(peeled):
                            pp, q0 = peeled[kp]
                        else:
                            pp = pp8p.tile([128, 2, 512], FP8, tag="pp8")
                            q0 = scores_pair(qc, h, kp, pp)
                        sums_av_pair(h, kp, npair, pp, q0, sums, avp)
                    finish_head(qc, h, sums, avp)

                def outproj(qc):
                    for sb in range(4 * qc, 4 * qc + 4):
                        ssl = slice(sb * 128, (sb + 1) * 128)
                        for oc2 in range(2):
                            op2 = accp.tile([128, 1024], F32, tag="acc")
                            o0 = slice((2 * oc2) * 512, (2 * oc2 + 1) * 512)
                            o1 = slice((2 * oc2 + 1) * 512, (2 * oc2 + 2) * 512)
                            if sb == 0:
                                for h in range(HPC):
                                    lhsT = aoT16[:, h, :]
                                    nc.tensor.matmul(
                                        op2[:, 0:512],
                                        lhsT,
                                        wo8[:, h, o0],
                                        start=(h == 0),
                                        stop=(h == HPC - 1),
                                    )
                                    nc.tensor.matmul(
                                        op2[:, 512:1024],
                                        lhsT,
                                        wo8[:, h, o1],
                                        start=(h == 0),
                                        stop=(h == HPC - 1),
                                    )
                            else:
                                for hp in range(2):
                                    hpsl = slice(2 * hp, 2 * hp + 2)
                                    lhsT = aoT8[:, hpsl, ssl]
                                    nc.tensor.matmul(
                                        op2[:, 0:512],
                                        lhsT,
                                        wo8[:, hpsl, o0],
                                        start=(hp == 0),
                                        stop=(hp == 1),
                                        perf_mode=DR,
                                    )
                                    nc.tensor.matmul(
                                        op2[:, 512:1024],
                                        lhsT,
                                        wo8[:, hpsl, o1],
                                        start=(hp == 0),
                                        stop=(hp == 1),
                                        perf_mode=DR,
                                    )
                            ob = osbp.tile([128, 1024], BF16, tag="osb")
                            if evict_flip[0] % 2 == 0:
                                nc.scalar.copy(ob[:, :], op2[:, :])
                                nc.scalar.dma_start(
                                    out=out_d[ssl, oc2 * 1024 : (oc2 + 1) * 1024],
                                    in_=ob[:, :],
                                )
                            else:
                                nc.vector.tensor_copy(ob[:, :], op2[:, :])
                                nc.sync.dma_start(
                                    out=out_d[ssl, oc2 * 1024 : (oc2 + 1) * 1024],
                                    in_=ob[:, :],
                                )
                            evict_flip[0] += 1

                for h in range(HPC):
                    attn_head0_clean(h)
                for qc in range(1, NQC):
                    # peel the next chunk's first scores so the PE has work
                    # while the previous chunk's aoT dependency chain drains.
                    peeled = []
                    for kp in range(2):
                        pp = pp8p.tile([128, 2, 512], FP8, tag="pp8")
                        q0 = scores_pair(qc, 0, kp, pp)
                        peeled.append((pp, q0))
                    outproj(qc - 1)
                    attn_head(qc, 0, peeled=peeled)
                    for h in range(1, HPC):
                        attn_head(qc, h)
                outproj(NQC - 1)

    nc.finalize()
    return nc


def _bit_quantize_ternary(w: np.ndarray):
    """Returns (ternary {-1,0,1} float32 matrix, scale) matching the reference."""
    scale = np.maximum(np.mean(np.abs(w.astype(np.float32))), np.float32(1e-5))
    t = np.clip(np.round(w.astype(np.float32) / scale), -1.0, 1.0).astype(np.float32)
    return t, float(scale)


def _host_tables():
    """cos/sin stacked [128, S]: rows 0:64 cos, rows 64:128 sin."""
    inv_freq = 1.0 / (ROPE_BASE ** (np.arange(0, D_HEAD, 2, dtype=np.float32) / D_HEAD))
    pos = np.arange(SEQ, dtype=np.float32)
    ang = pos[:, None] * inv_freq[None, :]  # [S, 64]
    cs = np.empty((128, SEQ), dtype=NPBF16)
    cs[0:64] = np.ascontiguousarray(np.cos(ang).T).astype(NPBF16)
    cs[64:128] = np.ascontiguousarray(np.sin(ang).T).astype(NPBF16)
    sc = np.empty((128, SEQ), dtype=NPBF16)
    sc[0:64] = cs[64:128]
    sc[64:128] = cs[0:64]
    return cs, sc


def kernel(x, w_qkv, w_out, mask):
    global LAST_RESULT
    x = np.asarray(x, dtype=np.float32)
    w_qkv = np.asarray(w_qkv, dtype=np.float32)
    w_out = np.asarray(w_out, dtype=np.float32)
    mask = np.asarray(mask)

    tq, sq = _bit_quantize_ternary(w_qkv)
    to, so = _bit_quantize_ternary(w_out)
    exp_scale = (sq * sq) / float(np.sqrt(D_HEAD))
    # wv is pre-scaled by 1/4 on upload (fp8 range); compensate here.
    c2 = np.float32(sq * so * 4.0)

    m2 = (mask.reshape(SEQ, SEQ) != 0).astype(np.float32)
    causal = bool(np.array_equal(m2, np.tril(np.ones((SEQ, SEQ), np.float32))))
    assert causal, "kernel specialized for the causal mask"

    cs, sc = _host_tables()
    maskd8 = np.empty((128, SEQ), dtype=NPFP8)
    for ki in range(NKI):
        blk = m2[ki * 128 : (ki + 1) * 128, ki * 128 : (ki + 1) * 128]  # [q, k]
        maskd8[:, ki * 128 : (ki + 1) * 128] = np.ascontiguousarray(blk.T).astype(
            NPFP8
        )
    maskd16 = maskd8[:, 0:CL].astype(NPBF16)

    key = float(exp_scale)
    if key not in _PROG_CACHE:
        _PROG_CACHE[key] = _build_program(float(exp_scale))
    nc = _PROG_CACHE[key]

    in_maps = []
    for c in range(N_CORES):
        b, g = divmod(c, 4)
        rows = slice(R_LOCAL * g, R_LOCAL * (g + 1))
        xT = np.ascontiguousarray(x[b].T)
        im = {
            "xT8": xT.astype(NPFP8),
            "xT16": np.ascontiguousarray(xT[:, 0:CL]).astype(NPBF16),
            "wqT": np.ascontiguousarray(tq[0 * D_MODEL :][rows].T).astype(NPFP8),
            "wkT": np.ascontiguousarray(tq[1 * D_MODEL :][rows].T).astype(NPFP8),
            "wvT": np.ascontiguousarray(tq[2 * D_MODEL :][rows].T * 0.25).astype(
                NPFP8
            ),
            "woT": np.ascontiguousarray(to[:, rows].T).astype(NPFP8),
            "cossinT": cs,
            "sincosT": sc,
            "maskd16": maskd16,
            "maskd8": maskd8,
        }
        in_maps.append(im)

    do_trace = bool(PROFILE) and _enable_profiling()
    res = run_bass_kernel_spmd(nc, in_maps, list(range(N_CORES)), trace=do_trace)
    LAST_RESULT = res

    parts = [np.asarray(res.results[c]["out"]).astype(np.float32) for c in range(N_CORES)]
    out = np.stack(
        [
            parts[0] + parts[1] + parts[2] + parts[3],
            parts[4] + parts[5] + parts[6] + parts[7],
        ]
    )
    return (out * c2).astype(np.float32)
